# revision 2
# baseline (speedup 1.0000x reference)
"""Trainium2 Bass kernel for 2-layer HypergraphConv (PyG-style), 8-core SPMD.

Sharding: A-phases (node->hyperedge segment sum) partitioned by hyperedge
(each core owns 25k hyperedges; its e-table is fully local); B-phases
(hyperedge->node) use the same entry shard, producing partial node sums over
all 100k nodes, combined with one AllReduce per layer. Weight matmuls are
folded around the segment sums (linearity), so every gather moves 256B rows.
Host work is index-only preprocessing (sort/block/pad + degree constants).
"""
import numpy as np

import concourse.bass as bass
import concourse.mybir as mybir
import concourse.tile as tile
from concourse.bass_utils import run_bass_kernel_spmd

f32 = mybir.dt.float32
i32 = mybir.dt.int32

N, M, E = 100000, 200000, 1600000
NC = 8
PB = 128
N_PAD = 100352            # 784 node blocks
NBLK = N_PAD // PB
N_TAB = N_PAD + PB        # x~/h1 gather tables get one extra zero block
ZROW_A = N_TAB - 1
M_LOC = M // NC
M_LOC_PAD = 25088         # 196 hyperedge blocks per core
MBLK = M_LOC_PAD // PB
E_TAB = M_LOC_PAD + PB    # e-table + zero pad block
ZROW_B = E_TAB - 1
SHARD_N = N_PAD // NC     # 12544
SHBLK = SHARD_N // PB     # 98


# ---------------------------------------------------------------------------
# patch: this walrus build supports only ONE sync-wait per instruction; hoist
# extra waits into standalone EventSemaphore instructions in the BIR JSON.
def _patch_split_waits():
    import json

    if getattr(bass.Bass, "_split_waits_patched", False):
        return
    orig = bass.Bass.to_json_bytes

    def to_json_bytes(self, *a, **k):
        raw = orig(self, *a, **k)
        m = json.loads(raw)
        ctr = 0
        changed = False
        for fn in m.get("functions", []):
            for bb in fn.get("blocks", []):
                insts = bb.get("instructions", [])
                out = []
                for ins in insts:
                    si = ins.get("sync_info")
                    if si and len(si.get("on_wait") or []) > 1:
                        for w in si["on_wait"][:-1]:
                            ctr += 1
                            out.append({
                                "debug": ins.get("debug", 0),
                                "engine": ins["engine"],
                                "ins": [],
                                "name": f"splitwait_{ctr}_{ins['name']}",
                                "opcode": "EventSemaphore",
                                "outs": [],
                                "sync_info": {"on_update": [], "on_wait": [w]},
                            })
                        si["on_wait"] = [si["on_wait"][-1]]
                        changed = True
                    out.append(ins)
                if changed:
                    bb["instructions"] = out
        return json.dumps(m).encode() if changed else raw

    bass.Bass.to_json_bytes = to_json_bytes
    bass.Bass._split_waits_patched = True


# ---------------------------------------------------------------------------
# host-side index preprocessing
def _tile_arrays(seg_local, gather_idx, blk_of_seg, n_blocks, ntiles, zero_row):
    order = np.argsort(blk_of_seg, kind="stable")
    sl, gi, blk = seg_local[order], gather_idx[order], blk_of_seg[order]
    starts = np.searchsorted(blk, np.arange(n_blocks), side="left")
    ends = np.searchsorted(blk, np.arange(n_blocks), side="right")
    gs, ss = [], []
    for b in range(n_blocks):
        s, e = starts[b], ends[b]
        pad = ntiles[b] * PB - (e - s)
        g = np.concatenate([gi[s:e], np.full(pad, zero_row, np.int64)])
        sv = np.concatenate([sl[s:e], np.zeros(pad, np.int64)])
        gs.append(g.reshape(ntiles[b], PB).T)
        ss.append(sv.reshape(ntiles[b], PB).T)
    return (np.concatenate(gs, 1).astype(np.int32),
            np.concatenate(ss, 1).astype(np.float32))


def preprocess(edge_index, edge_weight):
    node_idx = np.asarray(edge_index[0], np.int64)
    hedge_idx = np.asarray(edge_index[1], np.int64)
    w = np.asarray(edge_weight, np.float32)

    Bdeg = np.bincount(hedge_idx, minlength=M).astype(np.float32)
    Binv = np.where(Bdeg > 0, 1.0 / np.maximum(Bdeg, 0.5), 0.0)
    u = (w * Binv).astype(np.float32)
    D = np.zeros(N, np.float32)
    np.add.at(D, node_idx, w[hedge_idx])
    Dinv = np.where(D > 0, 1.0 / np.maximum(D, 1e-30), 0.0).astype(np.float32)

    per_core = []
    for c in range(NC):
        mask = (hedge_idx >= c * M_LOC) & (hedge_idx < (c + 1) * M_LOC)
        nd, hl = node_idx[mask], hedge_idx[mask] - c * M_LOC
        per_core.append((nd, hl))

    # unified per-block tile counts across cores (SPMD: identical structure)
    ntA = np.ones(MBLK, np.int64)
    ntB = np.ones(NBLK, np.int64)
    for nd, hl in per_core:
        ca = np.bincount(hl // PB, minlength=MBLK)
        cb = np.bincount(nd // PB, minlength=NBLK)
        ntA = np.maximum(ntA, (ca + PB - 1) // PB)
        ntB = np.maximum(ntB, (cb + PB - 1) // PB)

    cores = []
    for c, (nd, hl) in enumerate(per_core):
        gA, sA = _tile_arrays(hl % PB, nd, hl // PB, MBLK, ntA, ZROW_A)
        gB, sB = _tile_arrays(nd % PB, hl, nd // PB, NBLK, ntB, ZROW_B)
        u_loc = np.zeros(M_LOC_PAD, np.float32)
        u_loc[:M_LOC] = u[c * M_LOC:(c + 1) * M_LOC]
        cores.append(dict(gA=gA, sA=sA, gB=gB, sB=sB,
                          u_t=np.ascontiguousarray(
                              u_loc.reshape(MBLK, PB).T)))
    Dinv_pad = np.zeros(N_PAD, np.float32)
    Dinv_pad[:N] = Dinv
    dinv_t = np.ascontiguousarray(Dinv_pad.reshape(NBLK, PB).T)
    return cores, dinv_t, ntA, ntB


# ---------------------------------------------------------------------------
# The final shard selection needs per-core row offsets; implement via an
# extra per-core index input rsel [128, SHBLK] holding global row ids.
def build_module2(TA, TB, ntA, ntB):
    nc = bass.Bass(trn_type="TRN2")
    xT = nc.declare_dram_parameter("xT", [128, N_TAB], f32, isOutput=False)
    W1 = nc.declare_dram_parameter("W1", [128, 64], f32, isOutput=False)
    W2 = nc.declare_dram_parameter("W2", [64, 128], f32, isOutput=False)
    b1r = nc.declare_dram_parameter("b1r", [128, 64], f32, isOutput=False)
    b2r = nc.declare_dram_parameter("b2r", [128, 128], f32, isOutput=False)
    iota = nc.declare_dram_parameter("iota", [128, 128], f32, isOutput=False)
    gA = nc.declare_dram_parameter("gA", [128, TA], i32, isOutput=False)
    sA = nc.declare_dram_parameter("sA", [128, TA], f32, isOutput=False)
    gB = nc.declare_dram_parameter("gB", [128, TB], i32, isOutput=False)
    sB = nc.declare_dram_parameter("sB", [128, TB], f32, isOutput=False)
    u_t = nc.declare_dram_parameter("u_t", [128, MBLK], f32, isOutput=False)
    dinvs = nc.declare_dram_parameter("dinvs", [128, SHBLK], f32, isOutput=False)
    dinv = nc.declare_dram_parameter("dinv", [128, NBLK], f32, isOutput=False)
    rsel = nc.declare_dram_parameter("rsel", [128, SHBLK], i32, isOutput=False)
    out = nc.declare_dram_parameter("out", [SHARD_N, 128], f32, isOutput=True)

    mult = mybir.AluOpType.mult
    add = mybir.AluOpType.add
    maxop = mybir.AluOpType.max
    iseq = mybir.AluOpType.is_equal

    with tile.TileContext(nc) as tc:
        with (
            tc.tile_pool(name="const", bufs=1) as cp,
            tc.tile_pool(name="idx", bufs=1) as ip,
            tc.tile_pool(name="ld", bufs=4) as lp,
            tc.tile_pool(name="g", bufs=8) as gp,
            tc.tile_pool(name="sel", bufs=8) as sp,
            tc.tile_pool(name="blk", bufs=4) as bp,
            tc.tile_pool(name="ps", bufs=4, space="PSUM") as pp,
            tc.tile_pool(name="psf", bufs=2, space="PSUM") as pf,
            tc.tile_pool(name="dram", bufs=1, space="DRAM") as dp,
        ):
            W1t = cp.tile([128, 64], f32)
            W2t = cp.tile([64, 128], f32)
            b1t = cp.tile([128, 64], f32)
            b2t = cp.tile([128, 128], f32)
            iot = cp.tile([128, 128], f32)
            ut = cp.tile([128, MBLK], f32)
            dit = cp.tile([128, NBLK], f32)
            dst = cp.tile([128, SHBLK], f32)
            zt = cp.tile([128, 64], f32)
            idt = cp.tile([128, 128], f32)
            nc.sync.dma_start(out=W1t[:], in_=W1[:, :])
            nc.sync.dma_start(out=W2t[:], in_=W2[:, :])
            nc.sync.dma_start(out=b1t[:], in_=b1r[:, :])
            nc.sync.dma_start(out=b2t[:], in_=b2r[:, :])
            nc.sync.dma_start(out=iot[:], in_=iota[:, :])
            nc.sync.dma_start(out=ut[:], in_=u_t[:, :])
            nc.sync.dma_start(out=dit[:], in_=dinv[:, :])
            nc.sync.dma_start(out=dst[:], in_=dinvs[:, :])
            nc.vector.memset(zt[:], 0.0)
            from concourse.masks import make_identity
            make_identity(nc, idt[:])

            gAt = ip.tile([128, TA], i32)
            sAt = ip.tile([128, TA], f32)
            gBt = ip.tile([128, TB], i32)
            sBt = ip.tile([128, TB], f32)
            rst = ip.tile([128, SHBLK], i32)
            nc.sync.dma_start(out=gAt[:], in_=gA[:, :])
            nc.sync.dma_start(out=sAt[:], in_=sA[:, :])
            nc.sync.dma_start(out=gBt[:], in_=gB[:, :])
            nc.sync.dma_start(out=sBt[:], in_=sB[:, :])
            nc.sync.dma_start(out=rst[:], in_=rsel[:, :])

            xt1 = dp.tile([N_TAB, 64], f32)
            ets = dp.tile([E_TAB, 64], f32)
            h1 = dp.tile([N_TAB, 64], f32)
            cc1i = dp.tile([N_PAD, 64], f32)
            cc1o = dp.tile([N_PAD, 64], f32)
            cc2i = dp.tile([N_PAD, 64], f32)
            cc2o = dp.tile([N_PAD, 64], f32)

            for grp in range(NBLK // 4):
                ld = lp.tile([128, 512], f32, tag="xld")
                nc.sync.dma_start(out=ld[:], in_=xT[:, grp * 512:(grp + 1) * 512])
                for j in range(4):
                    b = grp * 4 + j
                    ps = pp.tile([128, 64], f32, tag="mm")
                    nc.tensor.matmul(out=ps[:], lhsT=ld[:, j * 128:(j + 1) * 128],
                                     rhs=W1t[:], start=True, stop=True)
                    ob = bp.tile([128, 64], f32, tag="ob")
                    nc.scalar.copy(out=ob[:], in_=ps[:])
                    nc.sync.dma_start(out=xt1[b * PB:(b + 1) * PB, :], in_=ob[:])
            nc.sync.dma_start(out=xt1[N_PAD:N_TAB, :], in_=zt[:])
            nc.sync.dma_start(out=ets[M_LOC_PAD:E_TAB, :], in_=zt[:])

            def seg_phase(table, gidx, sel_ids, ntiles, n_blocks, finish):
                t0 = 0
                for b in range(n_blocks):
                    ps = pp.tile([128, 64], f32, tag="mm")
                    for k in range(ntiles[b]):
                        col = t0 + k
                        g = gp.tile([128, 64], f32, tag="g")
                        nc.gpsimd.indirect_dma_start(
                            out=g[:], out_offset=None, in_=table[:, :],
                            in_offset=bass.IndirectOffsetOnAxis(
                                ap=gidx[:, col:col + 1], axis=0))
                        s = sp.tile([128, 128], f32, tag="sel")
                        nc.vector.tensor_tensor(
                            out=s[:],
                            in0=sel_ids[:, col:col + 1].to_broadcast([128, 128]),
                            in1=iot[:], op=iseq)
                        nc.tensor.matmul(out=ps[:], lhsT=s[:], rhs=g[:],
                                         start=(k == 0), stop=(k == ntiles[b] - 1))
                    t0 += ntiles[b]
                    finish(b, ps)

            def finA(b, ps):
                ob = bp.tile([128, 64], f32, tag="ob")
                nc.vector.tensor_tensor(out=ob[:], in0=ps[:],
                                        in1=ut[:, b:b + 1].to_broadcast([128, 64]),
                                        op=mult)
                nc.sync.dma_start(out=ets[b * PB:(b + 1) * PB, :], in_=ob[:])

            def mk_finB(dst_dram):
                def finB(b, ps):
                    ob = bp.tile([128, 64], f32, tag="ob")
                    nc.scalar.copy(out=ob[:], in_=ps[:])
                    nc.sync.dma_start(out=dst_dram[b * PB:(b + 1) * PB, :],
                                      in_=ob[:])
                return finB

            seg_phase(xt1, gAt, sAt, ntA, MBLK, finA)
            seg_phase(ets, gBt, sBt, ntB, NBLK, mk_finB(cc1i))
            nc.gpsimd.collective_compute(
                "AllReduce", add, replica_groups=[list(range(NC))],
                ins=[cc1i.opt()], outs=[cc1o.opt()])

            for b in range(NBLK):
                t = lp.tile([128, 64], f32, tag="h1ld")
                nc.sync.dma_start(out=t[:], in_=cc1o[b * PB:(b + 1) * PB, :])
                t2 = lp.tile([128, 64], f32, tag="h1t2")
                nc.vector.tensor_tensor(
                    out=t2[:], in0=t[:],
                    in1=dit[:, b:b + 1].to_broadcast([128, 64]), op=mult)
                nc.vector.tensor_tensor(out=t2[:], in0=t2[:], in1=b1t[:], op=add)
                nc.vector.tensor_relu(out=t2[:], in_=t2[:])
                nc.sync.dma_start(out=h1[b * PB:(b + 1) * PB, :], in_=t2[:])
            nc.sync.dma_start(out=h1[N_PAD:N_TAB, :], in_=zt[:])

            seg_phase(h1, gAt, sAt, ntA, MBLK, finA)
            seg_phase(ets, gBt, sBt, ntB, NBLK, mk_finB(cc2i))
            nc.gpsimd.collective_compute(
                "AllReduce", add, replica_groups=[list(range(NC))],
                ins=[cc2i.opt()], outs=[cc2o.opt()])

            # final: gather shard rows of cc2o via per-core row ids, scale by
            # Dinv, project by W2, bias+relu
            for b in range(SHBLK):
                t = gp.tile([128, 64], f32, tag="g")
                nc.gpsimd.indirect_dma_start(
                    out=t[:], out_offset=None, in_=cc2o[:, :],
                    in_offset=bass.IndirectOffsetOnAxis(
                        ap=rst[:, b:b + 1], axis=0))
                t2 = lp.tile([128, 64], f32, tag="fs")
                nc.vector.tensor_tensor(
                    out=t2[:], in0=t[:],
                    in1=dst[:, b:b + 1].to_broadcast([128, 64]), op=mult)
                psT = pf.tile([64, 128], f32, tag="psT")
                nc.tensor.matmul(out=psT[:], lhsT=t2[:], rhs=idt[:],
                                 start=True, stop=True)
                sT = lp.tile([64, 128], f32, tag="sT")
                nc.scalar.copy(out=sT[:], in_=psT[:])
                ps2 = pf.tile([128, 128], f32, tag="ps2")
                nc.tensor.matmul(out=ps2[:], lhsT=sT[:], rhs=W2t[:],
                                 start=True, stop=True)
                ob = bp.tile([128, 128], f32, tag="fo")
                nc.vector.tensor_tensor(out=ob[:], in0=ps2[:], in1=b2t[:], op=add)
                nc.vector.tensor_relu(out=ob[:], in_=ob[:])
                nc.sync.dma_start(out=out[b * PB:(b + 1) * PB, :], in_=ob[:])
    return nc


def kernel(x, edge_index, edge_weight, batch, W1, b1, W2, b2):
    _patch_split_waits()
    x = np.asarray(x, np.float32)
    W1 = np.asarray(W1, np.float32)
    b1 = np.asarray(b1, np.float32)
    W2 = np.asarray(W2, np.float32)
    b2 = np.asarray(b2, np.float32)

    cores, dinv_t, ntA, ntB = preprocess(np.asarray(edge_index),
                                         np.asarray(edge_weight))
    TA, TB = int(ntA.sum()), int(ntB.sum())

    xTp = np.zeros((128, N_TAB), np.float32)
    xTp[:, :N] = x.T
    iota = np.tile(np.arange(128, dtype=np.float32), (128, 1))
    b1r = np.tile(b1[None, :], (128, 1)).astype(np.float32)
    b2r = np.tile(b2[None, :], (128, 1)).astype(np.float32)

    nc = build_module2(TA, TB, ntA.tolist(), ntB.tolist())

    in_maps = []
    for c in range(NC):
        p = cores[c]
        rows = (c * SHARD_N
                + np.arange(SHARD_N).reshape(SHBLK, PB).T).astype(np.int32)
        dinvs = dinv_t[:, c * SHBLK:(c + 1) * SHBLK]
        in_maps.append({
            "xT": xTp, "W1": W1, "W2": W2, "b1r": b1r, "b2r": b2r,
            "iota": iota, "gA": p["gA"], "sA": p["sA"], "gB": p["gB"],
            "sB": p["sB"], "u_t": p["u_t"],
            "dinv": dinv_t, "dinvs": np.ascontiguousarray(dinvs),
            "rsel": np.ascontiguousarray(rows),
        })
    res = run_bass_kernel_spmd(nc, in_maps, core_ids=list(range(NC)))
    full = np.concatenate([res.results[c]["out"] for c in range(NC)], axis=0)
    return full[:N].astype(np.float32)


# revision 4
# speedup vs baseline: 1.7251x; 1.7251x over previous
"""Trainium2 Bass kernel for 2-layer HypergraphConv (PyG-style), 8-core SPMD.

Sharding: A-phases (node->hyperedge segment sum) partitioned by hyperedge
(each core owns 25k hyperedges; its e-table is fully local); B-phases
(hyperedge->node) use the same entry shard, producing partial node sums over
all 100k nodes, combined with one AllReduce per layer. Weight matmuls are
folded around the segment sums (linearity), so every gather moves 256B rows.
Host work is index-only preprocessing (sort/block/pad + degree constants).
"""
import numpy as np

import concourse.bass as bass
import concourse.mybir as mybir
import concourse.tile as tile
from concourse.bass_utils import run_bass_kernel_spmd

f32 = mybir.dt.float32
i32 = mybir.dt.int32

N, M, E = 100000, 200000, 1600000
NC = 8
PB = 128
N_PAD = 100352            # 784 node blocks
NBLK = N_PAD // PB
N_TAB = N_PAD + PB        # x~/h1 gather tables get one extra zero block
ZROW_A = N_TAB - 1
M_LOC = M // NC
M_LOC_PAD = 25088         # 196 hyperedge blocks per core
MBLK = M_LOC_PAD // PB
E_TAB = M_LOC_PAD + PB    # e-table + zero pad block
ZROW_B = E_TAB - 1
SHARD_N = N_PAD // NC     # 12544
SHBLK = SHARD_N // PB     # 98


# ---------------------------------------------------------------------------
# patch: this walrus build supports only ONE sync-wait per instruction; hoist
# extra waits into standalone EventSemaphore instructions in the BIR JSON.
def _patch_split_waits():
    import json

    if getattr(bass.Bass, "_split_waits_patched", False):
        return
    orig = bass.Bass.to_json_bytes

    def to_json_bytes(self, *a, **k):
        raw = orig(self, *a, **k)
        m = json.loads(raw)
        ctr = 0
        changed = False
        for fn in m.get("functions", []):
            for bb in fn.get("blocks", []):
                insts = bb.get("instructions", [])
                out = []
                for ins in insts:
                    si = ins.get("sync_info")
                    if si and len(si.get("on_wait") or []) > 1:
                        for w in si["on_wait"][:-1]:
                            ctr += 1
                            out.append({
                                "debug": ins.get("debug", 0),
                                "engine": ins["engine"],
                                "ins": [],
                                "name": f"splitwait_{ctr}_{ins['name']}",
                                "opcode": "EventSemaphore",
                                "outs": [],
                                "sync_info": {"on_update": [], "on_wait": [w]},
                            })
                        si["on_wait"] = [si["on_wait"][-1]]
                        changed = True
                    out.append(ins)
                if changed:
                    bb["instructions"] = out
        return json.dumps(m).encode() if changed else raw

    bass.Bass.to_json_bytes = to_json_bytes
    bass.Bass._split_waits_patched = True


# ---------------------------------------------------------------------------
# host-side index preprocessing
def _tile_arrays(seg_local, gather_idx, blk_of_seg, n_blocks, ntiles, zero_row):
    order = np.argsort(blk_of_seg, kind="stable")
    sl, gi, blk = seg_local[order], gather_idx[order], blk_of_seg[order]
    starts = np.searchsorted(blk, np.arange(n_blocks), side="left")
    ends = np.searchsorted(blk, np.arange(n_blocks), side="right")
    gs, ss = [], []
    for b in range(n_blocks):
        s, e = starts[b], ends[b]
        pad = ntiles[b] * PB - (e - s)
        g = np.concatenate([gi[s:e], np.full(pad, zero_row, np.int64)])
        sv = np.concatenate([sl[s:e], np.zeros(pad, np.int64)])
        gs.append(g.reshape(ntiles[b], PB).T)
        ss.append(sv.reshape(ntiles[b], PB).T)
    return (np.concatenate(gs, 1).astype(np.int32),
            np.concatenate(ss, 1).astype(np.float32))


def preprocess(edge_index, edge_weight):
    node_idx = np.asarray(edge_index[0], np.int64)
    hedge_idx = np.asarray(edge_index[1], np.int64)
    w = np.asarray(edge_weight, np.float32)

    Bdeg = np.bincount(hedge_idx, minlength=M).astype(np.float32)
    Binv = np.where(Bdeg > 0, 1.0 / np.maximum(Bdeg, 0.5), 0.0)
    u = (w * Binv).astype(np.float32)
    D = np.zeros(N, np.float32)
    np.add.at(D, node_idx, w[hedge_idx])
    Dinv = np.where(D > 0, 1.0 / np.maximum(D, 1e-30), 0.0).astype(np.float32)

    pnode = np.arange(N, dtype=np.int64)
    per_core = []
    for c in range(NC):
        mask = (hedge_idx >= c * M_LOC) & (hedge_idx < (c + 1) * M_LOC)
        nd, hl = node_idx[mask], hedge_idx[mask] - c * M_LOC
        per_core.append((nd, hl))

    # unified per-block tile counts across cores (SPMD: identical structure)
    ntA = np.ones(MBLK, np.int64)
    ntB = np.ones(NBLK, np.int64)
    for nd, hl in per_core:
        ca = np.bincount(hl // PB, minlength=MBLK)
        cb = np.bincount(nd // PB, minlength=NBLK)
        ntA = np.maximum(ntA, (ca + PB - 1) // PB)
        ntB = np.maximum(ntB, (cb + PB - 1) // PB)

    cores = []
    for c, (nd, hl) in enumerate(per_core):
        gA, sA = _tile_arrays(hl % PB, nd, hl // PB, MBLK, ntA, ZROW_A)
        gB, sB = _tile_arrays(nd % PB, hl, nd // PB, NBLK, ntB, ZROW_B)
        u_loc = np.zeros(M_LOC_PAD, np.float32)
        u_loc[:M_LOC] = u[c * M_LOC:(c + 1) * M_LOC]
        cores.append(dict(gA=gA, sA=sA, gB=gB, sB=sB,
                          u_t=np.ascontiguousarray(
                              u_loc.reshape(MBLK, PB).T)))
    Dinv_pad = np.zeros(N_PAD, np.float32)
    Dinv_pad[:N] = Dinv
    dinv_t = np.ascontiguousarray(Dinv_pad.reshape(NBLK, PB).T)
    return cores, dinv_t, ntA, ntB, pnode


# ---------------------------------------------------------------------------
# The final shard selection needs per-core row offsets; implement via an
# extra per-core index input rsel [128, SHBLK] holding global row ids.
def build_module2(TA, TB, ntA, ntB):
    nc = bass.Bass(trn_type="TRN2")
    xT = nc.declare_dram_parameter("xT", [128, N_TAB], f32, isOutput=False)
    W1 = nc.declare_dram_parameter("W1", [128, 64], f32, isOutput=False)
    W2 = nc.declare_dram_parameter("W2", [64, 128], f32, isOutput=False)
    b1r = nc.declare_dram_parameter("b1r", [128, 64], f32, isOutput=False)
    b2r = nc.declare_dram_parameter("b2r", [128, 128], f32, isOutput=False)
    iota = nc.declare_dram_parameter("iota", [128, 128], f32, isOutput=False)
    gA = nc.declare_dram_parameter("gA", [128, TA], i32, isOutput=False)
    sA = nc.declare_dram_parameter("sA", [128, TA], f32, isOutput=False)
    gB = nc.declare_dram_parameter("gB", [128, TB], i32, isOutput=False)
    sB = nc.declare_dram_parameter("sB", [128, TB], f32, isOutput=False)
    u_t = nc.declare_dram_parameter("u_t", [128, MBLK], f32, isOutput=False)
    dinvs = nc.declare_dram_parameter("dinvs", [128, SHBLK], f32, isOutput=False)
    dinv = nc.declare_dram_parameter("dinv", [128, NBLK], f32, isOutput=False)
    rsel = nc.declare_dram_parameter("rsel", [128, SHBLK], i32, isOutput=False)
    out = nc.declare_dram_parameter("out", [SHARD_N, 128], f32, isOutput=True)

    mult = mybir.AluOpType.mult
    add = mybir.AluOpType.add
    maxop = mybir.AluOpType.max
    iseq = mybir.AluOpType.is_equal

    with tile.TileContext(nc) as tc:
        with (
            tc.tile_pool(name="const", bufs=1) as cp,
            tc.tile_pool(name="idx", bufs=1) as ip,
            tc.tile_pool(name="ld", bufs=4) as lp,
            tc.tile_pool(name="g", bufs=8) as gp,
            tc.tile_pool(name="sel", bufs=8) as sp,
            tc.tile_pool(name="blk", bufs=4) as bp,
            tc.tile_pool(name="ps", bufs=4, space="PSUM") as pp,
            tc.tile_pool(name="psf", bufs=2, space="PSUM") as pf,
            tc.tile_pool(name="dram", bufs=1, space="DRAM") as dp,
        ):
            W1t = cp.tile([128, 64], f32)
            W2t = cp.tile([64, 128], f32)
            b1t = cp.tile([128, 64], f32)
            b2t = cp.tile([128, 128], f32)
            iot = cp.tile([128, 128], f32)
            ut = cp.tile([128, MBLK], f32)
            dit = cp.tile([128, NBLK], f32)
            dst = cp.tile([128, SHBLK], f32)
            zt = cp.tile([128, 64], f32)
            idt = cp.tile([128, 128], f32)
            nc.sync.dma_start(out=W1t[:], in_=W1[:, :])
            nc.sync.dma_start(out=W2t[:], in_=W2[:, :])
            nc.sync.dma_start(out=b1t[:], in_=b1r[:, :])
            nc.sync.dma_start(out=b2t[:], in_=b2r[:, :])
            nc.sync.dma_start(out=iot[:], in_=iota[:, :])
            nc.sync.dma_start(out=ut[:], in_=u_t[:, :])
            nc.sync.dma_start(out=dit[:], in_=dinv[:, :])
            nc.sync.dma_start(out=dst[:], in_=dinvs[:, :])
            nc.vector.memset(zt[:], 0.0)
            from concourse.masks import make_identity
            make_identity(nc, idt[:])

            gAt = ip.tile([128, TA], i32)
            sAt = ip.tile([128, TA], f32)
            gBt = ip.tile([128, TB], i32)
            sBt = ip.tile([128, TB], f32)
            rst = ip.tile([128, SHBLK], i32)
            nc.sync.dma_start(out=gAt[:], in_=gA[:, :])
            nc.sync.dma_start(out=sAt[:], in_=sA[:, :])
            nc.sync.dma_start(out=gBt[:], in_=gB[:, :])
            nc.sync.dma_start(out=sBt[:], in_=sB[:, :])
            nc.sync.dma_start(out=rst[:], in_=rsel[:, :])

            xt1 = dp.tile([N_TAB, 64], f32)
            ets = dp.tile([E_TAB, 64], f32)
            h1 = dp.tile([N_TAB, 64], f32)
            cc1i = dp.tile([N_PAD, 64], f32)
            cc1o = dp.tile([N_PAD, 64], f32)
            cc2i = dp.tile([N_PAD, 64], f32)
            cc2o = dp.tile([N_PAD, 64], f32)

            for grp in range(NBLK // 4):
                ld = lp.tile([128, 512], f32, tag="xld")
                nc.sync.dma_start(out=ld[:], in_=xT[:, grp * 512:(grp + 1) * 512])
                for j in range(4):
                    b = grp * 4 + j
                    ps = pp.tile([128, 64], f32, tag="mm")
                    nc.tensor.matmul(out=ps[:], lhsT=ld[:, j * 128:(j + 1) * 128],
                                     rhs=W1t[:], start=True, stop=True)
                    ob = bp.tile([128, 64], f32, tag="ob")
                    nc.scalar.copy(out=ob[:], in_=ps[:])
                    nc.sync.dma_start(out=xt1[b * PB:(b + 1) * PB, :], in_=ob[:])
            nc.sync.dma_start(out=xt1[N_PAD:N_TAB, :], in_=zt[:])
            nc.sync.dma_start(out=ets[M_LOC_PAD:E_TAB, :], in_=zt[:])

            def seg_phase(table, gidx, sel_ids, ntiles, n_blocks, finish):
                t0 = 0
                for b in range(n_blocks):
                    ps = pp.tile([128, 64], f32, tag="mm")
                    for k in range(ntiles[b]):
                        col = t0 + k
                        g = gp.tile([128, 64], f32, tag="g")
                        nc.gpsimd.indirect_dma_start(
                            out=g[:], out_offset=None, in_=table[:, :],
                            in_offset=bass.IndirectOffsetOnAxis(
                                ap=gidx[:, col:col + 1], axis=0))
                        s = sp.tile([128, 128], f32, tag="sel")
                        nc.vector.tensor_tensor(
                            out=s[:],
                            in0=sel_ids[:, col:col + 1].to_broadcast([128, 128]),
                            in1=iot[:], op=iseq)
                        nc.tensor.matmul(out=ps[:], lhsT=s[:], rhs=g[:],
                                         start=(k == 0), stop=(k == ntiles[b] - 1))
                    t0 += ntiles[b]
                    finish(b, ps)

            def finA(b, ps):
                ob = bp.tile([128, 64], f32, tag="ob")
                nc.vector.tensor_tensor(out=ob[:], in0=ps[:],
                                        in1=ut[:, b:b + 1].to_broadcast([128, 64]),
                                        op=mult)
                nc.sync.dma_start(out=ets[b * PB:(b + 1) * PB, :], in_=ob[:])

            def mk_finB(dst_dram):
                def finB(b, ps):
                    ob = bp.tile([128, 64], f32, tag="ob")
                    nc.scalar.copy(out=ob[:], in_=ps[:])
                    nc.sync.dma_start(out=dst_dram[b * PB:(b + 1) * PB, :],
                                      in_=ob[:])
                return finB

            seg_phase(xt1, gAt, sAt, ntA, MBLK, finA)
            seg_phase(ets, gBt, sBt, ntB, NBLK, mk_finB(cc1i))
            nc.gpsimd.collective_compute(
                "AllReduce", add, replica_groups=[list(range(NC))],
                ins=[cc1i.opt()], outs=[cc1o.opt()])

            for b in range(NBLK):
                t = lp.tile([128, 64], f32, tag="h1ld")
                nc.sync.dma_start(out=t[:], in_=cc1o[b * PB:(b + 1) * PB, :])
                t2 = lp.tile([128, 64], f32, tag="h1t2")
                nc.vector.tensor_tensor(
                    out=t2[:], in0=t[:],
                    in1=dit[:, b:b + 1].to_broadcast([128, 64]), op=mult)
                nc.vector.tensor_tensor(out=t2[:], in0=t2[:], in1=b1t[:], op=add)
                nc.vector.tensor_relu(out=t2[:], in_=t2[:])
                nc.sync.dma_start(out=h1[b * PB:(b + 1) * PB, :], in_=t2[:])
            nc.sync.dma_start(out=h1[N_PAD:N_TAB, :], in_=zt[:])

            seg_phase(h1, gAt, sAt, ntA, MBLK, finA)
            seg_phase(ets, gBt, sBt, ntB, NBLK, mk_finB(cc2i))
            nc.gpsimd.collective_compute(
                "AllReduce", add, replica_groups=[list(range(NC))],
                ins=[cc2i.opt()], outs=[cc2o.opt()])

            # final: gather shard rows of cc2o via per-core row ids, scale by
            # Dinv, project by W2, bias+relu
            for b in range(SHBLK):
                t = gp.tile([128, 64], f32, tag="g")
                nc.gpsimd.indirect_dma_start(
                    out=t[:], out_offset=None, in_=cc2o[:, :],
                    in_offset=bass.IndirectOffsetOnAxis(
                        ap=rst[:, b:b + 1], axis=0))
                t2 = lp.tile([128, 64], f32, tag="fs")
                nc.vector.tensor_tensor(
                    out=t2[:], in0=t[:],
                    in1=dst[:, b:b + 1].to_broadcast([128, 64]), op=mult)
                psT = pf.tile([64, 128], f32, tag="psT")
                nc.tensor.matmul(out=psT[:], lhsT=t2[:], rhs=idt[:],
                                 start=True, stop=True)
                sT = lp.tile([64, 128], f32, tag="sT")
                nc.scalar.copy(out=sT[:], in_=psT[:])
                ps2 = pf.tile([128, 128], f32, tag="ps2")
                nc.tensor.matmul(out=ps2[:], lhsT=sT[:], rhs=W2t[:],
                                 start=True, stop=True)
                ob = bp.tile([128, 128], f32, tag="fo")
                nc.vector.tensor_tensor(out=ob[:], in0=ps2[:], in1=b2t[:], op=add)
                nc.vector.tensor_relu(out=ob[:], in_=ob[:])
                nc.sync.dma_start(out=out[b * PB:(b + 1) * PB, :], in_=ob[:])
    return nc


_MODULE_CACHE = {}


def kernel(x, edge_index, edge_weight, batch, W1, b1, W2, b2):
    _patch_split_waits()
    x = np.asarray(x, np.float32)
    W1 = np.asarray(W1, np.float32)
    b1 = np.asarray(b1, np.float32)
    W2 = np.asarray(W2, np.float32)
    b2 = np.asarray(b2, np.float32)

    cores, dinv_t, ntA, ntB, pnode = preprocess(np.asarray(edge_index),
                                                np.asarray(edge_weight))
    TA, TB = int(ntA.sum()), int(ntB.sum())

    xTp = np.zeros((128, N_TAB), np.float32)
    xTp[:, :N] = x.T
    iota = np.tile(np.arange(128, dtype=np.float32), (128, 1))
    b1r = np.tile(b1[None, :], (128, 1)).astype(np.float32)
    b2r = np.tile(b2[None, :], (128, 1)).astype(np.float32)

    key = (TA, TB, tuple(ntA.tolist()), tuple(ntB.tolist()))
    nc = _MODULE_CACHE.get(key)
    if nc is None:
        nc = build_module2(TA, TB, ntA.tolist(), ntB.tolist())
        _MODULE_CACHE[key] = nc

    in_maps = []
    for c in range(NC):
        p = cores[c]
        rows = (c * SHARD_N
                + np.arange(SHARD_N).reshape(SHBLK, PB).T).astype(np.int32)
        dinvs = dinv_t[:, c * SHBLK:(c + 1) * SHBLK]
        in_maps.append({
            "xT": xTp, "W1": W1, "W2": W2, "b1r": b1r, "b2r": b2r,
            "iota": iota, "gA": p["gA"], "sA": p["sA"], "gB": p["gB"],
            "sB": p["sB"], "u_t": p["u_t"],
            "dinv": dinv_t, "dinvs": np.ascontiguousarray(dinvs),
            "rsel": np.ascontiguousarray(rows),
        })
    res = run_bass_kernel_spmd(nc, in_maps, core_ids=list(range(NC)))
    full = np.concatenate([res.results[c]["out"] for c in range(NC)], axis=0)
    return full[:N].astype(np.float32)
